# revision 64
# baseline (speedup 1.0000x reference)
"""Trainium2 Bass kernel for an AttnBlock (GroupNorm -> QKV 1x1 conv ->
spatial self-attention -> output projection -> residual).

Full-input contract: kernel(**inputs) takes the unsharded numpy inputs and
returns the full (4, 512, 64, 64) float32 output.

Sharding: 8 cores = 4 batches x 2 query-halves. Each core group-norms its
batch, runs attention for its 2048 queries over all 4096 keys, and writes
its query-half of the output. The per-core x input is column-rotated on the
host so that each core's own queries are always columns [0, 2048).

Algebraic fusions (exact up to rounding):
- scores: q_i.k_j = h_j^T (Wq^T Wk) h_i + (Wk^T bq).h_j + [j-constant terms
  dropped]. K/Q are never materialized; R = (Wq^T Wk)-contracted H_q with
  w2 = Wk^T bq folded in, so the exp bias is a plain constant.
- attention output: Wp (V P) = (Wp Wv) (H P) + Wp bv, so V is never
  materialized; softmax denominators divide out after the projection.

Numerics: every large matmul (scores, attention, R, denominator, output
projection) runs in fp8e4 with perf_mode=DoubleRow (K=256 per instruction,
0.5 cycles/row): 4x the bf16 contraction rate. Small fused weights are
pre-scaled by 16 to stay in fp8 normal range; the 16s cancel through the
softmax normalization. exp bias -3.0 guards fp8 overflow and cancels in
the normalization. All accumulation is f32 PSUM; GroupNorm statistics,
denominators, residual and epilogue stay f32.

Pipeline: per query chunk (icq) a 3-stage software pipeline runs:
  stage S (period k):   scores + exp -> E-matrix chunk (double-buffered)
  stage A (period k+1): denominator (front-loaded, rec ready mid-period)
                        + attention (two 2-channel-chunk passes)
  stage P (period k+2): output projection + epilogue + store
PSUM: 4 banks score pairs (x2 bufs), 2 attention, 1 denominator, 1 proj;
startup runs under its own left-stack pools (GN stats, W3/w2, transposes,
R) that hand their banks to the attention pools after period 0. Startup
work that is off the x->GroupNorm->R critical path (cc2/cc3 transposes,
R for icq 1-3, M2T, w4) is interleaved into period 0's score groups.
Engine balance: exp owns ACT; GroupNorm statistics own DVE during the x
stream (h8 writes ride ACT); weight casts ride Pool; the projection
epilogue (x16 rescale + 1/den + w4 + residual) is two fused DVE ops.
"""

from contextlib import ExitStack

import numpy as np

import concourse.mybir as mybir
import concourse.tile as tile
from concourse import bacc
from concourse.bass_utils import run_bass_kernel_spmd
import concourse.hw_specs as _hw_specs

# The static tile scheduler models every DMA as if it were alone on one
# engine's slice of bandwidth, which makes it believe the x chunks land
# ~4x later than they do and order the engine streams around phantom
# waits (head-blocking GroupNorm behind unrelated work). Schedule with
# aggregate DMA bandwidth instead; execution semantics are unaffected.
if not getattr(_hw_specs.TRN2Spec, "_attn_sched_dma_patched", False):
    _hw_specs.TRN2Spec.DMA_BUS_BYTES_PER_NS_PER_ENGINE = 360e9 / 1e9
    _hw_specs.TRN2Spec.DMA_CYCLE = _hw_specs.TRN2Spec.DMA_CYCLE / 4.0
    _hw_specs.TRN2Spec._attn_sched_dma_patched = True

# Problem geometry (hardcoded; the grading harness stages only kernel.py).
B = 4
C = 512
HW = 64
N = HW * HW          # 4096 keys per batch
NQ = N // 2          # 2048 queries per core
GROUPS = 32
GSIZE = C // GROUPS  # 16 channels per group
EPS = 1e-6

P = 128
CT = C // P          # 4 channel chunks
JT = N // P          # 32 key chunks of 128
NI = 512             # query tile
IC = NQ // NI        # 4 query chunks per core

F32 = mybir.dt.float32
BF16 = mybir.dt.bfloat16
FP8 = mybir.dt.float8e4

W16 = 16.0           # fp8 pre-scale for small fused weights
EXP_BIAS = -3.0

PARAM_NAMES = ("bq", "bk", "bv", "bp", "gn_scale", "gn_bias")
WEIGHT_NAMES = ("wq", "wk", "wv", "wp")

_BUILD_CACHE = {}


def _emit(ctx, nc, tc, x_d, w_d, p_d, out_d, repeat=1):
    AF = mybir.ActivationFunctionType
    ALU = mybir.AluOpType

    consts = ctx.enter_context(tc.tile_pool(name="consts", bufs=1))
    small = ctx.enter_context(tc.tile_pool(name="small", bufs=4))
    stage = ctx.enter_context(tc.tile_pool(name="stage", bufs=6))
    wnat = ctx.enter_context(tc.tile_pool(name="wnat", bufs=1))
    wf8 = ctx.enter_context(tc.tile_pool(name="wf8", bufs=1))
    hpool = ctx.enter_context(tc.tile_pool(name="hpool", bufs=1))
    epool = ctx.enter_context(tc.tile_pool(name="epool", bufs=2))
    opool = ctx.enter_context(tc.tile_pool(name="opool", bufs=2))

    for _rep in range(repeat):
        _emit_body(nc, tc, x_d, w_d, p_d, out_d, consts, small, stage,
                   wnat, wf8, hpool, epool, opool, AF, ALU, _rep)


def _emit_body(nc, tc, x_d, w_d, p_d, out_d, consts, small, stage,
               wnat, wf8, hpool, epool, opool, AF, ALU, rep):
    DR = mybir.MatmulPerfMode.DoubleRow
    inv16 = float(C) ** -0.5 / W16

    # Manual schedule stamps: the static tile scheduler orders each engine
    # stream by these monotone pseudo-times, pinning program order where its
    # own cost model would otherwise shuffle the startup.
    _stamp = [0]

    def stamp():
        _stamp[0] += 1
        tc.tile_set_cur_wait(_stamp[0] * 1e-6)

    # ---- constants -------------------------------------------------------
    # Pool-engine constants first: the transposes need `ident8` and nothing
    # should queue ahead of it on GpSimd.
    ident8 = consts.tile([P, P], FP8, tag="ident8")
    nc.gpsimd.memset(ident8, 0.0)
    nc.gpsimd.affine_select(
        out=ident8, in_=ident8, compare_op=ALU.not_equal, fill=1.0,
        base=0, pattern=[[-1, P]], channel_multiplier=1,
    )
    ones16 = consts.tile([P, 2, P], FP8, tag="ones16")
    nc.gpsimd.memset(ones16, W16)

    # Per-channel params as (128, CT): column cc = channels [cc*128, ..+128).
    par = {}
    for name in PARAM_NAMES:
        t = consts.tile([P, CT], F32, tag=f"par_{name}", name=f"par_{name}")
        nc.gpsimd.dma_start(out=t, in_=p_d[name][:].rearrange("(t p) -> p t", p=P))
        par[name] = t
    # Group-reduction matrices (see baseline): G averages 16 channels into
    # a group; GE expands group values back to channels.
    GPC = P // GSIZE
    gmat = consts.tile([P, GPC], F32, tag="gmat")
    nc.gpsimd.memset(gmat, 1.0 / GSIZE)
    nc.gpsimd.affine_select(
        out=gmat, in_=gmat, compare_op=ALU.is_ge, fill=0.0,
        base=0, pattern=[[-GSIZE, GPC]], channel_multiplier=1,
    )
    nc.gpsimd.affine_select(
        out=gmat, in_=gmat, compare_op=ALU.is_ge, fill=0.0,
        base=GSIZE - 1, pattern=[[GSIZE, GPC]], channel_multiplier=-1,
    )
    gexp = consts.tile([GPC, P], F32, tag="gexp")
    nc.gpsimd.memset(gexp, 1.0)
    nc.gpsimd.affine_select(
        out=gexp, in_=gexp, compare_op=ALU.is_ge, fill=0.0,
        base=0, pattern=[[1, P]], channel_multiplier=-GSIZE,
    )
    nc.gpsimd.affine_select(
        out=gexp, in_=gexp, compare_op=ALU.is_ge, fill=0.0,
        base=GSIZE - 1, pattern=[[-1, P]], channel_multiplier=GSIZE,
    )
    eps8 = consts.tile([GPC, 1], F32, tag="eps8")
    nc.vector.memset(eps8, EPS)
    expb = consts.tile([P, 1], F32, tag="expb")
    nc.vector.memset(expb, EXP_BIAS)

    # ---- x + weights DMA, ordered for earliest h8: x0/x1 first, then
    # wq/wk (needed by W3 -> R0), x2/x3 inside the GN loop, wv/wp last.
    # Weights ride the ACT HWDGE ring; x rides the SP ring. bf16 weight
    # casts on Pool.
    xs_t = {}
    for cc in (0, 1):
        xs_t[cc] = stage.tile([P, N], F32, tag="xstage",
                              name=f"xs_{rep}_{cc}", bufs=3)
        nc.sync.dma_start(out=xs_t[cc], in_=x_d[cc * P:(cc + 1) * P, :])

    w_nat = {}

    def load_w(wname):
        w_nat[wname] = wnat.tile([P, CT, C], BF16, tag=f"wn_{wname}",
                                 name=f"wn_{wname}")
        ws = stage.tile([P, CT, C], F32, tag="wstage",
                        name=f"ws_{rep}_{wname}", bufs=2)
        nc.scalar.dma_start(
            out=ws, in_=w_d[wname][:].rearrange("(t p) c -> p t c", p=P))
        nc.gpsimd.tensor_copy(out=w_nat[wname], in_=ws)

    load_w("wq")
    load_w("wk")
    bq_bf = consts.tile([P, CT], BF16, tag="bq_bf")
    nc.vector.tensor_copy(out=bq_bf, in_=par["bq"])
    bv_bf = consts.tile([P, CT], BF16, tag="bv_bf")
    nc.vector.tensor_copy(out=bv_bf, in_=par["bv"])

    h8 = hpool.tile([P, CT, N], FP8, tag="h8")
    ht = hpool.tile([P, JT, C], FP8, tag="ht")
    r8 = hpool.tile([P, CT, NQ], FP8, tag="r8")
    w3_8 = wf8.tile([P, CT, C], FP8, tag="w3_8")
    m2t_8 = wf8.tile([P, CT, C], FP8, tag="m2t_8")
    wv8 = wf8.tile([P, CT, C], FP8, tag="wv8")
    wp8 = wf8.tile([P, CT, C], FP8, tag="wp8")
    w2_16 = consts.tile([P, CT], F32, tag="w2_16")
    w4 = consts.tile([P, CT], F32, tag="w4")

    ps_tp0 = tc.alloc_tile_pool(name="ps_tp0", bufs=1, space="PSUM",
                                side="left")
    ps_r = tc.alloc_tile_pool(name="ps_r", bufs=2, space="PSUM", side="left")
    ps_tp = tc.alloc_tile_pool(name="ps_tp", bufs=1, space="PSUM",
                               side="left")
    ps_gn = tc.alloc_tile_pool(name="ps_gn", bufs=1, space="PSUM",
                               side="left")
    ps_w = tc.alloc_tile_pool(name="ps_w", bufs=2, space="PSUM", side="left")

    def emit_w3w2():
        # W3 = Wq^T Wk (b=c_q rows-chunks, a=c_k free), scaled x16 -> fp8.
        # Emitted mid-GN so it fills the PE idle window while x streams in.
        for bt in range(CT):
            ps = ps_w.tile([P, C], F32, tag="w")
            for co in range(CT):
                nc.tensor.matmul(
                    ps, lhsT=w_nat["wq"][:, co, bt * P:(bt + 1) * P],
                    rhs=w_nat["wk"][:, co, :],
                    start=(co == 0), stop=(co == CT - 1))
            nc.scalar.activation(out=w3_8[:, bt, :], in_=ps,
                                 func=AF.Identity, scale=W16)
        # w2_16 = 16 * Wk^T bq (folded into R during eviction).
        for at in range(CT):
            ps = ps_w.tile([P, C], F32, tag="w")
            for co in range(CT):
                nc.tensor.matmul(
                    ps[:, 0:1], lhsT=w_nat["wk"][:, co, at * P:(at + 1) * P],
                    rhs=bq_bf[:, co:co + 1],
                    start=(co == 0), stop=(co == CT - 1))
            nc.vector.tensor_scalar_mul(out=w2_16[:, at:at + 1],
                                        in0=ps[:, 0:1], scalar1=W16)

    with tc.high_priority():
        # ---- x load + GroupNorm -> h8 (fp8), transposes -> ht ------------
        # (cc3's transposes are deferred into period 0 below so the first
        # score matmuls aren't head-blocked behind them on PE.)
        for cc in range(CT):
            stamp()
            stats = small.tile([P, 8, 6], F32, tag="gn_stats",
                               name=f"gn_stats_{rep}_{cc}")
            if cc in xs_t:
                xs = xs_t.pop(cc)
            else:
                xs = stage.tile([P, N], F32, tag="xstage",
                                name=f"xs_{rep}_{cc}", bufs=3)
                nc.sync.dma_start(out=xs, in_=x_d[cc * P:(cc + 1) * P, :])
            for sg in range(8):
                nc.vector.bn_stats(out=stats[:, sg, :],
                                   in_=xs[:, sg * NI:(sg + 1) * NI])
            mv = small.tile([P, 2], F32, tag="gn_mv")
            nc.vector.bn_aggr(out=mv, in_=stats)
            # stat2 = [mean_c, E[x^2]_c]
            stat2 = small.tile([P, 2], F32, tag="gn_stat2")
            nc.vector.tensor_copy(out=stat2[:, 0:1], in_=mv[:, 0:1])
            nc.vector.tensor_scalar(
                out=stat2[:, 1:2], in0=mv[:, 0:1], scalar1=mv[:, 0:1],
                scalar2=mv[:, 1:2], op0=ALU.mult, op1=ALU.add)
            gnp = ps_gn.tile([P, 4], F32, tag="gn_acc")
            g_ps = gnp[0:GPC, 0:2]
            nc.tensor.matmul(g_ps, lhsT=gmat, rhs=stat2, start=True, stop=True)
            g_sb = small.tile([GPC, 2], F32, tag="gn_gsb")
            nc.vector.tensor_copy(out=g_sb, in_=g_ps)
            grp = small.tile([GPC, 2], F32, tag="gn_grp")
            nc.vector.tensor_copy(out=grp[:, 0:1], in_=g_sb[:, 0:1])
            nvar = small.tile([GPC, 1], F32, tag="gn_nvar")
            nc.vector.tensor_scalar(
                out=nvar, in0=g_sb[:, 0:1], scalar1=g_sb[:, 0:1],
                scalar2=g_sb[:, 1:2], op0=ALU.mult, op1=ALU.subtract)
            sd = small.tile([GPC, 1], F32, tag="gn_sd")
            nc.scalar.activation(out=sd, in_=nvar, func=AF.Sqrt, bias=eps8,
                                 scale=-1.0)
            nc.vector.reciprocal(out=grp[:, 1:2], in_=sd)
            e_ps = gnp[:, 2:4]
            nc.tensor.matmul(e_ps, lhsT=gexp, rhs=grp, start=True, stop=True)
            e_sb = small.tile([P, 2], F32, tag="gn_esb")
            nc.vector.tensor_copy(out=e_sb, in_=e_ps)
            # a_c = gn_scale * rstd ; nb_c = mean * a_c - gn_bias
            a_c = small.tile([P, 1], F32, tag="gn_a")
            nc.vector.tensor_mul(out=a_c, in0=par["gn_scale"][:, cc:cc + 1],
                                 in1=e_sb[:, 1:2])
            nb_c = small.tile([P, 1], F32, tag="gn_nb")
            nc.vector.tensor_scalar(
                out=nb_c, in0=e_sb[:, 0:1], scalar1=a_c,
                scalar2=par["gn_bias"][:, cc:cc + 1],
                op0=ALU.mult, op1=ALU.subtract)
            b_c = small.tile([P, 1], F32, tag="gn_b")
            nc.vector.tensor_scalar_mul(out=b_c, in0=nb_c, scalar1=-1.0)
            # h8 = a_c * x - nb_c, split across DVE and ACT halves for
            # every chunk (balances the startup streams; DVE takes the
            # first half, ACT the second).
            if False:
                nc.scalar.activation(
                    out=h8[:, cc, :], in_=xs, func=AF.Identity,
                    scale=a_c, bias=b_c)
            else:
                nc.vector.tensor_scalar(
                    out=h8[:, cc, :N // 2], in0=xs[:, :N // 2], scalar1=a_c,
                    scalar2=nb_c, op0=ALU.mult, op1=ALU.subtract)
                nc.scalar.activation(
                    out=h8[:, cc, N // 2:], in_=xs[:, N // 2:],
                    func=AF.Identity, scale=a_c, bias=b_c)
            # hT blocks: 8 transposes packed per PSUM bank (fp8 transpose
            # writes at element step 2). cc0/cc1 here with ACT evictions
            # (DVE stays stats-only); cc2/cc3 are deferred into period 0
            # with DVE evictions.
            if cc >= 2:
                continue
            for pk in range(4):
                tp = ps_tp.tile([P, 8, P, 2], FP8, tag="tp",
                                name=f"tp_{rep}_{cc}_{pk}")
                for k in range(8):
                    jc = pk * 8 + k
                    nc.tensor.matmul(
                        tp[:, k, :, 0], lhsT=h8[:, cc, jc * P:(jc + 1) * P],
                        rhs=ident8, is_transpose=True, skip_group_check=True)
                dst = ht[:, pk * 8:(pk + 1) * 8, cc * P:(cc + 1) * P]
                nc.scalar.activation(out=dst, in_=tp[:, :, :, 0],
                                     func=AF.Identity)
        stamp()
        emit_w3w2()
        load_w("wv")
        load_w("wp")

    def emit_r(icq):
        # R = x16-scaled (Wq^T Wk)-contraction of H_q, + w2, fp8.
        for at in range(CT):
            ps = ps_r.tile([P, NI], F32, tag="r")
            for m in range(2):
                nc.tensor.matmul(
                    ps, lhsT=w3_8[:, 2 * m:2 * m + 2, at * P:(at + 1) * P],
                    rhs=h8[:, 2 * m:2 * m + 2, icq * NI:(icq + 1) * NI],
                    start=(m == 0), stop=(m == 1), perf_mode=DR,
                    skip_group_check=True)
            nc.vector.tensor_scalar_add(
                out=r8[:, at, icq * NI:(icq + 1) * NI], in0=ps,
                scalar1=w2_16[:, at:at + 1])

    stamp()
    with tc.high_priority():
        emit_r(0)
    ps_w.release()
    ps_gn.release()
    ps_tp.release()

    # ---- 3-stage pipelined attention over query chunks -------------------
    # Period 0 runs with ps_tp/ps_r still open (deferred startup tail work
    # interleaves with its score/exp groups); the attention-side pools open
    # afterwards on the banks ps_tp/ps_r vacate.
    ps_s = tc.alloc_tile_pool(name="ps_s", bufs=2, space="PSUM",
                              side="right")

    e2f = {}       # kS -> E-matrix tile (pool-rotated, 2 live)
    att_sb = {}    # kA -> fp8 attention accumulators
    rec = {}       # kA -> 1/(16*den)
    xr = {}        # kP -> residual x chunk

    def emit_scores(kS, g):
        sps = ps_s.tile([P, 2, NI], F32, tag="sps",
                        name=f"sps_{rep}_{kS}_{g}")
        for u in range(2):
            jc = 2 * g + u
            for m in range(2):
                nc.tensor.matmul(
                    sps[:, u, :],
                    lhsT=h8[:, 2 * m:2 * m + 2, jc * P:(jc + 1) * P],
                    rhs=r8[:, 2 * m:2 * m + 2, kS * NI:(kS + 1) * NI],
                    start=(m == 0), stop=(m == 1), perf_mode=DR,
                    skip_group_check=True)
        nc.scalar.activation(
            out=e2f[kS][:, 2 * g:2 * g + 2, :], in_=sps,
            func=AF.Exp, scale=inv16, bias=expb)

    # ---- period 0: scores(0)/exp(0) + deferred startup tail --------------
    e2f[0] = epool.tile([P, JT, NI], FP8, tag="e2f", name=f"e2f_{rep}_0")
    tpd = {}
    for g in range(16):
        stamp()
        emit_scores(0, g)
        if g < 8:
            # cc2 + cc3 transposes: one 8-wide pack per cc per 2 groups,
            # evicted on DVE (idle in period 0).
            pk, half = g // 2, g % 2
            for cc in (2, 3):
                if half == 0:
                    tpd[cc] = ps_tp0.tile([P, 8, P, 2], FP8, tag=f"tp{cc}",
                                          name=f"tp_{rep}_{cc}_{pk}")
                tp = tpd[cc]
                for k in range(half * 4, half * 4 + 4):
                    jc = pk * 8 + k
                    nc.tensor.matmul(
                        tp[:, k, :, 0], lhsT=h8[:, cc, jc * P:(jc + 1) * P],
                        rhs=ident8, is_transpose=True, skip_group_check=True)
                if half == 1:
                    nc.vector.tensor_copy(
                        out=ht[:, pk * 8:(pk + 1) * 8, cc * P:(cc + 1) * P],
                        in_=tp[:, :, :, 0])
        elif g in (8, 10, 12):
            emit_r(g // 2 - 3)
        elif g == 9:
            # fp8 x16 copies of wv / wpT feeding the fp8-DR M2T matmuls.
            nc.vector.tensor_scalar_mul(out=wv8, in0=w_nat["wv"],
                                        scalar1=W16)
            nc.vector.tensor_scalar_mul(out=wp8, in0=w_nat["wp"],
                                        scalar1=W16)
        elif g in (13, 14):
            # M2T = (Wp Wv)^T (a=c_attn rows-chunks, d=c_out free), x16 fp8.
            # Computed as fp8 DoubleRow from x16-scaled operands (psum =
            # 256*M2T), rescaled by 1/16 on eviction.
            for at in (0, 1) if g == 13 else (2, 3):
                ps = ps_r.tile([P, NI], F32, tag="r")
                for m in range(2):
                    nc.tensor.matmul(
                        ps, lhsT=wv8[:, 2 * m:2 * m + 2, at * P:(at + 1) * P],
                        rhs=wp8[:, 2 * m:2 * m + 2, :],
                        start=(m == 0), stop=(m == 1), perf_mode=DR,
                        skip_group_check=True)
                nc.scalar.activation(out=m2t_8[:, at, :], in_=ps,
                                     func=AF.Identity, scale=1.0 / W16)
        elif g == 15:
            # w4 = Wp bv + bp (f32, added in the epilogue).
            for dt_ in range(CT):
                ps = ps_r.tile([P, NI], F32, tag="r")
                for ec in range(CT):
                    nc.tensor.matmul(
                        ps[:, 0:1],
                        lhsT=w_nat["wp"][:, ec, dt_ * P:(dt_ + 1) * P],
                        rhs=bv_bf[:, ec:ec + 1],
                        start=(ec == 0), stop=(ec == CT - 1))
                nc.vector.tensor_add(out=w4[:, dt_:dt_ + 1], in0=ps[:, 0:1],
                                     in1=par["bp"][:, dt_:dt_ + 1])

    ps_r.release()
    ps_tp0.release()
    ps_att = tc.alloc_tile_pool(name="ps_att", bufs=1, space="PSUM",
                                side="left")
    ps_den = tc.alloc_tile_pool(name="ps_den", bufs=1, space="PSUM",
                                side="left")
    ps_pp = tc.alloc_tile_pool(name="ps_pp", bufs=1, space="PSUM",
                               side="left")

    if True:
        att_ab = None
        den_ps = None

        for k in range(1, IC + 1):
            kS, kA, kP = k, k - 1, k - 2
            has_s = kS < IC
            has_a = 0 <= kA < IC
            has_p = 0 <= kP < IC

            if has_s:
                e2f[kS] = epool.tile([P, JT, NI], FP8, tag="e2f",
                                     name=f"e2f_{rep}_{kS}")
            if has_a:
                att_sb[kA] = opool.tile([P, CT, NI], FP8, tag="att_sb",
                                        name=f"attsb_{rep}_{kA}", bufs=2)
                rec[kA] = opool.tile([P, NI], F32, tag="rec",
                                     name=f"rec_{rep}_{kA}", bufs=2)
            if has_p:
                xr[kP] = opool.tile([P, CT, NI], F32, tag="xres",
                                    name=f"xr_{rep}_{kP}", bufs=2)
                nc.sync.dma_start(
                    out=xr[kP], in_=x_d[:, kP * NI:(kP + 1) * NI].rearrange(
                        "(t p) n -> p t n", p=P))

            for g in range(16):
                stamp()
                if has_s:
                    emit_scores(kS, g)

                if has_a:
                    if g == 0:
                        den_ps = ps_den.tile([P, NI], F32, tag="den",
                                             name=f"den_{rep}_{kA}")
                    # denominator compressed into g 0..7 (2 pairs/group) so
                    # rec is ready by g8, shortening the epilogue chain.
                    if g < 8:
                        for dj in range(2):
                            gg = 2 * g + dj
                            nc.tensor.matmul(
                                den_ps, lhsT=ones16,
                                rhs=e2f[kA][:, 2 * gg:2 * gg + 2, :],
                                start=(gg == 0), stop=(gg == 15),
                                perf_mode=DR, skip_group_check=True)
                    elif g == 8:
                        nc.vector.reciprocal(out=rec[kA], in_=den_ps)
                    # attention: pass A (ct 0,1) over g 0..7, pass B (ct
                    # 2,3) over g 8..15; each pass covers all 16 jp pairs.
                    half, gh = g // 8, g % 8
                    if gh == 0:
                        # For the last chunk the score banks are idle (no
                        # scores this period): run pass B there so both
                        # passes proceed concurrently instead of
                        # serializing on the pass-A eviction.
                        pool, tag = ((ps_s, "sps")
                                     if (kA == IC - 1 and half == 1)
                                     else (ps_att, "att"))
                        att_ab = pool.tile([P, 2, NI], F32, tag=tag,
                                           name=f"att_{rep}_{kA}_{half}")
                    for jj in range(2):
                        jp = 2 * gh + jj
                        for ci in range(2):
                            ct = 2 * half + ci
                            nc.tensor.matmul(
                                att_ab[:, ci, :],
                                lhsT=ht[:, 2 * jp:2 * jp + 2,
                                        ct * P:(ct + 1) * P],
                                rhs=e2f[kA][:, 2 * jp:2 * jp + 2, :],
                                start=(jp == 0), stop=(jp == 15),
                                perf_mode=DR, skip_group_check=True)
                    if gh == 7:
                        nc.vector.tensor_copy(
                            out=att_sb[kA][:, 2 * half:2 * half + 2, :],
                            in_=att_ab)

                if has_p and g % 4 == 3:
                    dc = g // 4
                    pp = ps_pp.tile([P, NI], F32, tag="pp",
                                    name=f"pp_{rep}_{kP}_{dc}")
                    for m in range(2):
                        nc.tensor.matmul(
                            pp, lhsT=m2t_8[:, 2 * m:2 * m + 2,
                                           dc * P:(dc + 1) * P],
                            rhs=att_sb[kP][:, 2 * m:2 * m + 2, :],
                            start=(m == 0), stop=(m == 1), perf_mode=DR,
                            skip_group_check=True)
                    tmp = opool.tile([P, NI], F32, tag="tmp", bufs=2,
                                     name=f"tmp_{rep}_{kP}_{dc}")
                    nc.vector.tensor_mul(out=tmp, in0=pp, in1=rec[kP])
                    ob = opool.tile([P, NI], F32, tag="ob", bufs=3,
                                    name=f"ob_{rep}_{kP}_{dc}")
                    nc.vector.scalar_tensor_tensor(
                        out=ob, in0=tmp, scalar=w4[:, dc:dc + 1],
                        in1=xr[kP][:, dc, :], op0=ALU.add, op1=ALU.add)
                    nc.sync.dma_start(
                        out=out_d[dc * P:(dc + 1) * P,
                                  kP * NI:(kP + 1) * NI], in_=ob)

            if has_a:
                att_sb.pop(kA - 2, None)
                rec.pop(kA - 2, None)
            if has_p:
                e2f.pop(kP, None)
                xr.pop(kP - 1, None)

        # ---- drain: projection for the last query chunk ------------------
        # The first half of each projection contraction (ct pair 0) only
        # needs att pass A, so it issues before the pass-B eviction lands;
        # two PSUM banks (pp + den, both free now) run two dc chains in
        # parallel.
        kP = IC - 1
        xr[kP] = opool.tile([P, CT, NI], F32, tag="xres",
                            name=f"xr_{rep}_{kP}", bufs=2)
        nc.sync.dma_start(
            out=xr[kP], in_=x_d[:, kP * NI:(kP + 1) * NI].rearrange(
                "(t p) n -> p t n", p=P))
        pps = {}
        for dc in range(CT):
            pool = ps_pp if dc % 2 == 0 else ps_den
            pps[dc] = pool.tile([P, NI], F32, tag="pp" if dc % 2 == 0
                                else "den", name=f"pp_{rep}_{kP}_{dc}")
            nc.tensor.matmul(
                pps[dc], lhsT=m2t_8[:, 0:2, dc * P:(dc + 1) * P],
                rhs=att_sb[kP][:, 0:2, :], start=True, stop=False,
                perf_mode=DR, skip_group_check=True)
        for dc in range(CT):
            nc.tensor.matmul(
                pps[dc], lhsT=m2t_8[:, 2:4, dc * P:(dc + 1) * P],
                rhs=att_sb[kP][:, 2:4, :], start=False, stop=True,
                perf_mode=DR, skip_group_check=True)
            tmp = opool.tile([P, NI], F32, tag="tmp", bufs=2,
                             name=f"tmp_{rep}_{kP}_{dc}")
            nc.vector.tensor_mul(out=tmp, in0=pps[dc], in1=rec[kP])
            ob = opool.tile([P, NI], F32, tag="ob", bufs=3,
                            name=f"ob_{rep}_{kP}_{dc}")
            nc.vector.scalar_tensor_tensor(
                out=ob, in0=tmp, scalar=w4[:, dc:dc + 1],
                in1=xr[kP][:, dc, :], op0=ALU.add, op1=ALU.add)
            nc.sync.dma_start(
                out=out_d[dc * P:(dc + 1) * P,
                          kP * NI:(kP + 1) * NI], in_=ob)

    ps_pp.release()
    ps_den.release()
    ps_att.release()
    ps_s.release()
    tc.tile_update_base_wait()


def _build(repeat=1):
    nc = bacc.Bacc()
    x_d = nc.declare_dram_parameter("x", [C, N], F32, isOutput=False)
    w_d = {w: nc.declare_dram_parameter(w, [C, C], F32, isOutput=False)
           for w in WEIGHT_NAMES}
    p_d = {p: nc.declare_dram_parameter(p, [C], F32, isOutput=False)
           for p in PARAM_NAMES}
    out_d = nc.declare_dram_parameter("out", [C, NQ], F32, isOutput=True)
    with tile.TileContext(nc) as tc, ExitStack() as ctx:
        _emit(ctx, nc, tc, x_d, w_d, p_d, out_d, repeat=repeat)
    nc.finalize()
    return nc


def _get_nc():
    if "nc" not in _BUILD_CACHE:
        _BUILD_CACHE["nc"] = _build()
    return _BUILD_CACHE["nc"]


def _make_in_maps(x, gn_scale, gn_bias, wq, bq, wk, bk, wv, bv, wp, bp):
    xf = np.ascontiguousarray(np.asarray(x, dtype=np.float32).reshape(B, C, N))
    shared = {
        "wq": np.ascontiguousarray(np.asarray(wq, np.float32)),
        "wk": np.ascontiguousarray(np.asarray(wk, np.float32)),
        "wv": np.ascontiguousarray(np.asarray(wv, np.float32)),
        # wp ships pre-transposed: the kernel wants c_in on rows.
        "wp": np.ascontiguousarray(np.asarray(wp, np.float32).T),
        "bq": np.ascontiguousarray(np.asarray(bq, np.float32)),
        "bk": np.ascontiguousarray(np.asarray(bk, np.float32)),
        "bv": np.ascontiguousarray(np.asarray(bv, np.float32)),
        "bp": np.ascontiguousarray(np.asarray(bp, np.float32)),
        "gn_scale": np.ascontiguousarray(np.asarray(gn_scale, np.float32)),
        "gn_bias": np.ascontiguousarray(np.asarray(gn_bias, np.float32)),
    }
    in_maps = []
    for core in range(8):
        bi, qh = core // 2, core % 2
        xb = xf[bi]
        if qh == 0:
            xc = xb
        else:
            xc = np.ascontiguousarray(
                np.concatenate([xb[:, NQ:], xb[:, :NQ]], axis=1))
        in_maps.append({"x": xc, **shared})
    return in_maps


def _gather(results):
    out = np.empty((B, C, N), np.float32)
    for core in range(8):
        bi, qh = core // 2, core % 2
        out[bi, :, qh * NQ:(qh + 1) * NQ] = results[core]["out"]
    return out.reshape(B, C, HW, HW)


def kernel(x, gn_scale, gn_bias, wq, bq, wk, bk, wv, bv, wp, bp):
    nc = _get_nc()
    in_maps = _make_in_maps(x, gn_scale, gn_bias, wq, bq, wk, bk, wv, bv,
                            wp, bp)
    res = run_bass_kernel_spmd(nc, in_maps, core_ids=list(range(8)))
    return _gather(res.results)
